# revision 40
# baseline (speedup 1.0000x reference)
"""Trainium2 Bass kernel for multi-head causal attention (v2).

Problem: q, k, v of shape [4096, 16, 64] (seq, heads, head_dim) fp32.
  out = softmax(causal(q @ k^T / 8)) @ v, reshaped to [4096, 1024].

Sharding: heads are split across 8 NeuronCores (2 heads per core).
Each core runs the same SPMD Bass program on its own 2 heads; the host
concatenates the per-core [4096, 128] outputs along the feature dim.

Per-core algorithm (S^T orientation, PE/ACT-balanced design):
  - Inputs stream in as fp32 via the two HWDGE queues (k on sync, q on
    scalar), are cast to bf16 on the DVE, then PE-transposed into
    qT/kT [(h,d)=128, 4096] lazily as each chunk is first needed
    (borrowing main-loop PSUM slots).  V loads via SWDGE cast DMA into
    vplus [128, 32*65]: each 128-row k-block gets 64 V columns plus a
    ones column (fused softmax denominator).
  - q groups (512 wide) are processed HEAVIEST FIRST (G=7..0): the
    causal triangle's thin front would otherwise idle both engines
    for ~20us at the start.  Per G, k blocks j <= 4G+3 in groups of 3;
    the 4 diagonal blocks form one column-packed group (order t0,t1,
    t3,t2 so no matmul output crosses a PSUM bank boundary) so the
    exp never touches masked columns:
      mm1:  S^T[kj, qi] = kT_j^T.T @ qT_G per head (PE row tiling)
      exp:  one ScalarE Exp per (jgroup, head), PSUM -> SBUF bf16;
            heads ping-pong across the two PSUM slots so ACT never
            waits on mm1
      mask: diagonal 128x128 triangle chunks multiplied by a 0/1 mask
      mm2 (reversed): O^T[d(+ones), qi] += vplus_j.T @ expS^T_j —
            V is the 65-column stationary so the PE streams each exp
            block once instead of 4x 128-col LDWEIGHTS per block
  - End of G: copy O^T [65, 512] to SBUF bf16, PE-transpose back into
    the freed PSUM bank ([128, 4*66]: out chunk ++ denominator col),
    reciprocal + row-scale on DVE, one batched output DMA per G.

No distributed primitives are needed: sharding is purely host-side.
"""

import numpy as np

SEQ = 4096
NHEAD = 16
HDIM = 64
NCORES = 8
HPC = NHEAD // NCORES  # heads per core = 2
SCALE = 0.125

# Schraudolph fast-exp constants (DVE path): exp(s*SCALE) is approximated by
# bitcast_bf16(int16(s * A_FE + B_FE)) — bf16 is the top 16 bits of fp32, so
# the classic 2^23-scaled trick divided by 2^16 lands the exponent+mantissa
# directly in an int16.  The piecewise-linear mantissa error is one-sided in
# [0, +6.15%] and largely cancels in the softmax ratio (validated end-to-end:
# rel err ~1e-2 if ALL blocks use it; we use it on ~35%).
LOG2E = 1.4426950408889634
A_FE = SCALE * LOG2E * (2.0 ** 7)
B_FE = 127.0 * (2.0 ** 7)

_NC_CACHE = {}
LAST_RESULT = {}


def build_attention_nc(seq=SEQ, hpc=HPC, hdim=HDIM):
    """Build the SPMD Bass program for one core handling `hpc` heads."""
    import concourse.bass as bass
    import concourse.mybir as mybir
    import concourse.tile as tile

    f32 = mybir.dt.float32
    bf16 = mybir.dt.bfloat16
    i16 = mybir.dt.int16
    Exp = mybir.ActivationFunctionType.Exp

    assert hpc == 2 and hdim == 64, "layout hardcoded for 2 heads x 64 dim"
    assert seq % 1024 == 0
    nt = seq // 128   # 128-row seq tiles = 32
    ng = seq // 512   # 512-wide q groups = 8
    nchunks = nt // 8  # staging chunks of 8 tiles = 4
    W = 80            # V block width incl ones column at 64, padded to a
                      # multiple of 16 so the DMA xbar can transpose O^T

    nc = bass.Bass()
    q = nc.dram_tensor("q", [seq, hpc, hdim], f32, kind="ExternalInput").ap()
    k = nc.dram_tensor("k", [seq, hpc, hdim], f32, kind="ExternalInput").ap()
    v = nc.dram_tensor("v", [seq, hpc, hdim], f32, kind="ExternalInput").ap()
    o = nc.dram_tensor("o", [seq, hpc * hdim], f32, kind="ExternalOutput").ap()

    with tile.TileContext(nc) as tc:
        with (
            tc.tile_pool(name="persist", bufs=1) as persist,
            tc.tile_pool(name="stage", bufs=6) as stage_pool,
            tc.tile_pool(name="pexp", bufs=6) as pexp_pool,
            tc.tile_pool(name="pexpD", bufs=6) as pexpD_pool,
            tc.tile_pool(name="oTs", bufs=2) as oTs_pool,
            tc.tile_pool(name="trp", bufs=2) as trp_pool,
            tc.tile_pool(name="outp", bufs=2) as out_pool,
            tc.tile_pool(name="small", bufs=8) as small_pool,
            tc.tile_pool(name="psum_s", bufs=3, space="PSUM") as ps_pool,
            tc.tile_pool(name="psum_o", bufs=1, space="PSUM") as po_pool,
        ):
            # ---- persistent SBUF tensors ----------------------------------
            qT = persist.tile([128, seq], bf16, tag="qT")
            kT = persist.tile([128, seq], bf16, tag="kT")
            vplus = [
                persist.tile([128, nt * W], bf16, tag=f"vplus{h}", name=f"vplus{h}")
                for h in range(hpc)
            ]
            # 0/1 lower-triangle mask for the in-chunk diagonal:
            # tri[kj, qi'] = 1 iff kj <= qi'  (same for every diagonal block)
            tri = persist.tile([128, 128], bf16, tag="tri")
            ident_b = persist.tile([128, 128], bf16, tag="identb")
            ident_f = persist.tile([128, 128], f32, tag="identf")

            from concourse.masks import make_identity

            # ---- K/Q loads: HWDGE fp32 (k on sync, q on scalar);
            # bf16 cast happens lazily at transpose time (tr_batch).
            # Loads are in first-use order for the descending-G loop;
            # the first pieces are 4 tiles so the first exp's DMA gate
            # is a 0.5MB transfer instead of 1MB.
            # (tile_start, ntiles) in load order:
            K_PIECES = [(0, 4), (4, 4), (8, 4), (12, 4), (16, 8), (24, 8)]
            Q_PIECES = [(28, 4), (24, 4), (16, 8), (8, 8), (0, 8)]
            # q piece first needed by each group G:
            QP_OF_G = {7: 0, 6: 1, 5: 2, 4: 2, 3: 3, 2: 3, 1: 4, 0: 4}
            stage_k = [None] * len(K_PIECES)
            stage_q = [None] * len(Q_PIECES)
            kp_of_tile = {}
            for pi, (t0, nt_) in enumerate(K_PIECES):
                for t in range(t0, t0 + nt_):
                    kp_of_tile[t] = pi

            def load_piece(src_t, eng, lst, pieces, pi):
                t0, nt_ = pieces[pi]
                st = stage_pool.tile(
                    [128, 8 * 128], f32, tag="st", name="st",
                    padded_shape=[128, 8 * 128],
                )[:, 0 : nt_ * 128]
                eng.dma_start(
                    out=st.rearrange("p (t x) -> p t x", x=128),
                    in_=src_t[t0 * 128 : (t0 + nt_) * 128, :, :].rearrange(
                        "(t p) h d -> p t (h d)", p=128
                    ),
                )
                lst[pi] = st

            # Preload the exp table + warm the PE clock while the first
            # DMAs are in flight: a dummy 1-col exp pulls the ~1.3us
            # ACT_TABLE_LOAD off the critical path, and ~3us of dummy
            # matmuls into the (idle until mm2) po slot release the HAM
            # clock gate (1.2 -> 2.4 GHz) before real PE work arrives.
            # Emitted FIRST so the DVE memset isn't queued behind the
            # DMA-waiting casts on the in-order vector queue.
            wsrc = persist.tile([128, 128], bf16, tag="wsrc", name="wsrc")
            wout = persist.tile([128, 1], bf16, tag="wout", name="wout")
            nc.vector.memset(wsrc, 0.0)
            wps = po_pool.tile([128, 512], f32, tag="po0", name="wps")
            for _ in range(80):
                nc.tensor.matmul(
                    wps[:, 0:128], lhsT=wsrc, rhs=wsrc,
                    start=True, stop=True, skip_group_check=True,
                )

            load_piece(k, nc.sync, stage_k, K_PIECES, 0)
            load_piece(q, nc.scalar, stage_q, Q_PIECES, 0)
            # dummy exp AFTER the q dispatch so it doesn't delay the DMA
            nc.scalar.activation(
                out=wout, in_=wsrc[:, 0:1], func=Exp, scale=1.0
            )
            # k is needed far earlier than q (the heaviest-first G loop
            # consumes all of kT within ~25us); stripe k across BOTH HWDGE
            # queues in small pieces so each k tile-pair lands before the
            # jgroup that streams it, and queue q1..q4 behind.
            load_piece(k, nc.scalar, stage_k, K_PIECES, 1)
            load_piece(k, nc.sync, stage_k, K_PIECES, 2)
            load_piece(k, nc.scalar, stage_k, K_PIECES, 3)
            load_piece(k, nc.sync, stage_k, K_PIECES, 4)
            load_piece(k, nc.scalar, stage_k, K_PIECES, 5)
            for pi in range(1, len(Q_PIECES)):
                load_piece(q, nc.scalar, stage_q, Q_PIECES, pi)
            # V after the k/q loads: it isn't needed until the first mm2
            # (~25us in), and its 2MB of HBM reads would otherwise slow
            # the first-exp-gating k0/q3 transfers
            for h in range(hpc):
                nc.vector.memset(vplus[h], 1.0)
                nc.gpsimd.dma_start(
                    out=vplus[h].rearrange("p (t x) -> p t x", x=W)[:, :, 0:hdim],
                    in_=v[:, h, :].rearrange("(t p) d -> p t d", p=128),
                )


            make_identity(nc, ident_b[:])
            make_identity(nc, ident_f[:])
            nc.vector.memset(tri, 1.0)
            nc.gpsimd.affine_select(
                out=tri[:],
                in_=tri[:],
                compare_op=mybir.AluOpType.is_ge,
                fill=0.0,
                base=0,
                pattern=[[1, 128]],
                channel_multiplier=-1,
            )

            # ---- main pipeline --------------------------------------------

            def tr_batch(st, dstT, t0, nt_):
                """Transpose one staged fp32 piece's seq-tiles into dstT
                columns, borrowing a main-loop PSUM slot as f32 scratch.
                No pre-cast: the PE transposes fp32 directly and the ACT
                copy-back does the bf16 cast, so neither the DVE nor the
                gpsimd queue sits on the PE's critical path."""
                trs = ps_pool.tile([128, 1024], f32, tag="ps", name="trs")
                for t in range(nt_):
                    nc.tensor.transpose(
                        trs[:, t * 128 : (t + 1) * 128],
                        st[:, t * 128 : (t + 1) * 128],
                        ident_f[:],
                    )
                nc.scalar.copy(
                    dstT[:, t0 * 128 : (t0 + nt_) * 128], trs[:, 0 : nt_ * 128]
                )

            def emit_mm2s(pend):
                """Deferred reversed-orientation P@V for one jgroup."""
                G, items, po, pes, njs = pend[:5]
                for h in range(hpc):
                    for j, moff, width, qcol in items:
                        nc.tensor.matmul(
                            po[h][:, qcol : qcol + width],
                            lhsT=vplus[h][:, j * W : (j + 1) * W],
                            rhs=pes[h][:, moff : moff + width],
                            start=(j == 0),
                            stop=(j == njs - 1),
                            skip_group_check=True,
                        )

            def emit_finals(G, po):
                oTs = []
                for h in range(hpc):
                    ot = oTs_pool.tile([W, 512], bf16, tag="oTs", name="oTs")
                    nc.scalar.copy(ot, po[h][:, :])
                    oTs.append(ot)
                WP = W  # chunk stride 80 cols = 160 B (already 4B-aligned)
                trh, recs = [], []
                for h in range(hpc):
                    # transpose O^T chunks back on the DMA xbar engine (sync
                    # queue, otherwise idle) instead of the PE: saves ~17us
                    # of PE time, and trp in SBUF lets the DVE ob-scale run
                    # in a faster perf mode than from PSUM.
                    trp = trp_pool.tile(
                        [128, 4 * WP], bf16, tag=f"trp{h}", name=f"tr{h}"
                    )
                    # one batched xbar transpose: [W, 512] -> logical [512, W]
                    # written as [128, 4, W] (chunk dim folded into partition).
                    # For the tail Gs (pipeline drained, queues idle) split
                    # h0/h1 across both HWDGE queues to halve the serial tail.
                    teng = nc.scalar if (G <= 1 and h == 1) else nc.sync
                    teng.dma_start_transpose(
                        trp.rearrange("p (c w) -> p c w", w=WP)[:, :, 0:W],
                        oTs[h][:, :],
                    )
                    rec = small_pool.tile([128, 4], f32, tag="rec", name="rec")
                    nc.vector.reciprocal(
                        rec, trp.rearrange("p (c x) -> p c x", x=WP)[:, :, hdim]
                    )
                    trh.append(trp)
                    recs.append(rec)
                ob = out_pool.tile([128, 4 * hpc * hdim], f32, tag="ob", name="ob")
                for h in range(hpc):
                    # one batched scale per head: rec broadcast along d via a
                    # stride-0 AP dim (4 ops/G instead of 8 - DVE op overhead
                    # is ~270ns regardless of size)
                    nc.vector.tensor_tensor(
                        out=ob.rearrange("p (c x) -> p c x", x=128)[
                            :, :, h * hdim : (h + 1) * hdim
                        ],
                        in0=trh[h].rearrange("p (c w) -> p c w", w=WP)[
                            :, :, 0:hdim
                        ],
                        in1=recs[h][:].broadcast_to([128, 4, hdim]),
                        op=mybir.AluOpType.mult,
                    )
                oeng = nc.scalar if G == 0 else nc.sync
                oeng.dma_start(
                    out=o[G * 512 : (G + 1) * 512, :].rearrange(
                        "(c p) d -> p c d", p=128
                    ),
                    in_=ob.rearrange("p (c d) -> p c d", d=hpc * hdim),
                )

            k_tr_done, q_tr_done = set(), set()

            def ensure_tr(done, stages, pieces, dstT, pi):
                if pi not in done:
                    done.add(pi)
                    tr_batch(stages[pi], dstT, *pieces[pi])

            # Heaviest groups first: the causal triangle's thin front
            # (G0..G2) otherwise idles both engines at the start, and the
            # light G1/G0 at the end keep the serial tail short.
            pending = None
            for G in reversed(range(ng)):
                njs = 4 * G + 4
                po = [
                    po_pool.tile([W, 512], f32, tag=f"po{h}", name=f"po{h}")
                    for h in range(hpc)
                ]
                ensure_tr(q_tr_done, stage_q, Q_PIECES, qT, QP_OF_G[G])

                # jgroups: (items, mask_offs).  Off-diagonal blocks in pairs
                # (1024-wide ps tiles = 2 PSUM banks, so ps_pool can hold 3
                # buffers and mm1 of jgroup i+1 never waits for exp of i);
                # the 4 diagonal blocks form two packed pairs.
                jgroups = [
                    (
                        [(j, (j - s) * 512, 512, 0) for j in range(s, min(s + 2, 4 * G))],
                        None,
                    )
                    for s in range(0, 4 * G, 2)
                ]
                jgroups.append(
                    (
                        [(4 * G + 0, 0, 512, 0), (4 * G + 1, 512, 384, 128)],
                        [0, 512],
                    )
                )
                jgroups.append(
                    (
                        [(4 * G + 2, 0, 256, 256), (4 * G + 3, 256, 128, 384)],
                        [0, 256],
                    )
                )
                for gi, (items, mask_offs) in enumerate(jgroups):
                    ensure_tr(k_tr_done, stage_k, K_PIECES, kT, kp_of_tile[items[-1][0]])
                    width = max(moff + wd for _, moff, wd, _ in items)
                    ps = [
                        ps_pool.tile([128, 1024], f32, tag="ps", name="ps")
                        for _ in range(hpc)
                    ]
                    for j, moff, wd, qcol in items:
                        for h in range(hpc):
                            nc.tensor.matmul(
                                ps[h][:, moff : moff + wd],
                                lhsT=kT[h * 64 : (h + 1) * 64, j * 128 : (j + 1) * 128],
                                rhs=qT[
                                    h * 64 : (h + 1) * 64,
                                    G * 512 + qcol : (G + 1) * 512,
                                ],
                                start=True,
                                stop=True,
                                tile_position=(h * 64, 0),
                            )
                    # exp path split BY HEAD: h0 on ACT (true exp), h1 on the
                    # DVE via Schraudolph fast-exp (one tensor_scalar:
                    # i16(s*A+B) whose bit pattern IS bf16 exp).  Running the
                    # two heads' exps on two engines in PARALLEL (instead of
                    # serially on one) keeps the per-jgroup exp latency under
                    # the PE's own per-jgroup work, so the PE never
                    # micro-stalls on exp — stalls let the HAM clock gate
                    # re-throttle the PE to 1.2 GHz for ~40us mid-kernel.
                    pes = []
                    for h in range(hpc):
                        if h == 1:
                            pd = pexpD_pool.tile(
                                [128, 1024], i16, tag="pexpD", name="pexpD"
                            )
                            nc.vector.tensor_scalar(
                                out=pd[:, 0:width],
                                in0=ps[h][:, 0:width],
                                scalar1=A_FE,
                                scalar2=B_FE,
                                op0=mybir.AluOpType.mult,
                                op1=mybir.AluOpType.add,
                            )
                            pes.append(pd[:].bitcast(bf16))
                        else:
                            pe = pexp_pool.tile(
                                [128, 1024], bf16, tag="pexp", name="pexp"
                            )
                            nc.scalar.activation(
                                out=pe[:, 0:width],
                                in_=ps[h][:, 0:width],
                                func=Exp,
                                scale=SCALE,
                            )
                            pes.append(pe)
                    if mask_offs is not None:  # diagonal pair: triangle masks
                        for h in range(hpc):
                            for mo in mask_offs:
                                nc.gpsimd.tensor_mul(
                                    pes[h][:, mo : mo + 128],
                                    pes[h][:, mo : mo + 128],
                                    tri[:],
                                )
                    if pending is not None:
                        emit_mm2s(pending)
                        if pending[5]:
                            emit_finals(pending[0], pending[2])
                    pending = (G, items, po, pes, njs, gi == len(jgroups) - 1)
            if pending is not None:
                emit_mm2s(pending)
                emit_finals(pending[0], pending[2])
    _split_multi_waits(nc)
    return nc


def _split_multi_waits(nc):
    """Walrus's codegen accepts at most one sync-wait per instruction on
    this toolchain. Hoist extra waits into standalone single-wait NoOps on
    the same engine queue (same semantics: the sequencer stalls in order)."""
    import concourse.mybir as mybir

    nsplit = 0
    for blk in nc.m.functions[0].blocks:
        newl = []
        for ins in blk.instructions:
            si = getattr(ins, "sync_info", None)
            if si is not None and si.on_wait and len(si.on_wait) > 1:
                waits = list(si.on_wait)
                for w in waits[:-1]:
                    newl.append(
                        mybir.InstNoOp(
                            name=f"{ins.name}-wsplit{nsplit}",
                            sync_info=mybir.SyncInfo(on_wait=[w], on_update=[]),
                            bass_nofuse=True,
                            engine=ins.engine,
                            ins=[],
                            outs=[],
                        )
                    )
                    nsplit += 1
                ins.sync_info = mybir.SyncInfo(
                    on_wait=[waits[-1]], on_update=list(si.on_update or [])
                )
            newl.append(ins)
        blk.instructions = newl
    return nsplit


def _ensure_ntff_hook():
    """The image's antenv package lacks axon_hooks; provide it so
    run_bass_kernel_spmd's trace path works (or degrades gracefully)."""
    import sys
    import types

    try:
        import antenv.axon_hooks  # noqa: F401

        return
    except ImportError:
        pass
    mod = types.ModuleType("antenv.axon_hooks")
    state = {"hook": None}
    mod.set_axon_ntff_profile_hook = lambda h: state.__setitem__("hook", h)
    mod.get_axon_ntff_profile_hook = lambda: state["hook"]
    try:
        from trn_agent_boot.trn_boot import _ntff_profile_via_ctypes

        state["hook"] = _ntff_profile_via_ctypes("/opt/axon/libaxon_pjrt.so")
    except Exception:
        state["hook"] = None
    sys.modules["antenv.axon_hooks"] = mod


def kernel(q, k, v):
    """Full-input entry point: q, k, v [4096, 16, 64] fp32 -> [4096, 1024]."""
    import sys

    if "/opt/trn_rl_repo" not in sys.path:
        sys.path.insert(0, "/opt/trn_rl_repo")
    _ensure_ntff_hook()
    from concourse.bass_utils import run_bass_kernel_spmd

    q = np.asarray(q, dtype=np.float32)
    k = np.asarray(k, dtype=np.float32)
    v = np.asarray(v, dtype=np.float32)
    seq, nhead, hdim = q.shape

    if "nc" not in _NC_CACHE:
        _NC_CACHE["nc"] = build_attention_nc(seq=seq, hpc=HPC, hdim=hdim)
    nc = _NC_CACHE["nc"]

    in_maps = []
    for c in range(NCORES):
        hs = slice(c * HPC, (c + 1) * HPC)
        in_maps.append(
            {
                "q": np.ascontiguousarray(q[:, hs, :]),
                "k": np.ascontiguousarray(k[:, hs, :]),
                "v": np.ascontiguousarray(v[:, hs, :]),
            }
        )
    res = run_bass_kernel_spmd(nc, in_maps, core_ids=list(range(NCORES)))
    LAST_RESULT["exec_time_ns"] = res.exec_time_ns
    try:
        iat = res.instructions_and_trace
        LAST_RESULT["trace_path"] = iat[1] if iat else None
    except Exception:
        LAST_RESULT["trace_path"] = None
    outs = [res.results[c]["o"] for c in range(NCORES)]
    return np.concatenate(outs, axis=1)

